# revision 3
# baseline (speedup 1.0000x reference)
"""Trainium2 Bass kernel for nn_LocalTrans (gnn message passing), v2.

Math (reference, exact simplifications):
  k = f@kw ; v = f@vw + vb                 (kb/qw/qb/pos drop out exactly:
                                            softmax offsets cancel, and ctx
                                            is invariant to per-(i,c) scaling
                                            of e, so kb's per-channel factor
                                            cancels)
  e = exp(-k[idx]/8) ; s = sum_K e
  ctx = (1/s) * max_K (e - s) * (v[idx])
  h = ctx@fw ; BatchNorm over B*N (fb cancels inside BN) ; LeakyReLU(0.2)
  out = f + h

Layout: channel-major "stacked". Each core owns 8192 nodes, split into
half-A (first 4096) and half-B. SBUF/PSUM partition p<64 holds channel p
of half-A data, p>=64 channel p-64 of half-B, so every engine runs at
full 128-partition width. Host gathers f[idx] (input re-layout only) and
streams it as fp16 [128, G, K, 512]; the PE keeps kw/vw stationary
(weights duplicated on both partition halves) and streams columns
through quadrant tile_position matmuls ((0,0) for A, (64,64) for B),
producing e/v channel-major in PSUM. Act does exp at full width; DVE
does the K sum-tree / subtract / max-tree in fp16 2x mode in-place; the
W = (v+vb)*D step is split: half the j-pairs are one scalar_tensor_tensor
from PSUM on DVE (vb rides as the per-partition scalar), the other half
are copied psum->sbuf by Act (vb as activation bias) and multiplied on
gpsimd. h = ctx@fw and the BN sum/sumsq partials run inside the main
loop; one [128,2] AllReduce, then a per-partition-affine Act + DVE
leaky-relu + gpsimd residual add finish it.

Sharding: 8 cores; core c -> batch c//2, node-half c%2 (8192 nodes).
"""

import sys

if "/opt/trn_rl_repo" not in sys.path:
    sys.path.insert(0, "/opt/trn_rl_repo")

import numpy as np
from contextlib import ExitStack

import concourse.bass as bass
import concourse.bacc as bacc
import concourse.tile as tile
from concourse import mybir
from concourse.bass_utils import run_bass_kernel_spmd

F32 = mybir.dt.float32
F16 = mybir.dt.float16
AF = mybir.ActivationFunctionType
OP = mybir.AluOpType
AX = mybir.AxisListType

B, N, C, K = 4, 16384, 64, 16
N_CORES = 8
NODES = N // 2          # nodes per core
HALF = NODES // 2       # 4096: nodes per stacked half
NG = 512                # nodes per group (per half)
GROUPS = HALF // NG     # 8
EPS = 1e-5
ALPHA = 0.2
M_TOT = float(B * N)

# per j-pair engine route for W = (v+vb)*D:
#   "stt"  -> DVE scalar_tensor_tensor direct from PSUM (1x)
#   "adve" -> Act copies v (+vb bias) to SBUF, DVE multiplies at 2x
#   "agps" -> Act copies v (+vb bias) to SBUF, gpsimd multiplies
W_MODES = ("stt", "agps", "stt", "agps", "stt", "agps", "stt", "agps")


def _build_program(n_cores=N_CORES, do_collective=True):
    nc = bacc.Bacc(None)

    fg_in = nc.dram_tensor("fg", [128, GROUPS * K * NG], F16, kind="ExternalInput")
    featT_in = nc.dram_tensor("featT", [128, HALF], F16, kind="ExternalInput")
    kw_in = nc.dram_tensor("kw", [128, C], F16, kind="ExternalInput")
    vw_in = nc.dram_tensor("vw", [128, C], F16, kind="ExternalInput")
    fw_in = nc.dram_tensor("fw", [128, C], F16, kind="ExternalInput")
    vbc_in = nc.dram_tensor("vbc", [128, 1], F32, kind="ExternalInput")
    gammac_in = nc.dram_tensor("gammac", [128, 1], F32, kind="ExternalInput")
    betac_in = nc.dram_tensor("betac", [128, 1], F32, kind="ExternalInput")
    fold128_in = nc.dram_tensor("fold128", [128, C], F32, kind="ExternalInput")
    fold64_in = nc.dram_tensor("fold64", [C, 128], F32, kind="ExternalInput")

    out_dram = nc.dram_tensor("out", [128, HALF], F16, kind="ExternalOutput")

    cc_in = nc.dram_tensor("cc_in", [128, 4], F32)
    cc_out = nc.dram_tensor("cc_out", [128, 4], F32, addr_space="Shared")
    ccw_in = nc.dram_tensor("ccw_in", [1, 1], F32)
    ccw_out = nc.dram_tensor("ccw_out", [1, 1], F32, addr_space="Shared")

    fg_view = fg_in[:, :].rearrange("p (g j n) -> p g j n", g=GROUPS, j=K)

    with tile.TileContext(nc) as tc:
        with ExitStack() as ctx:
            cpool = ctx.enter_context(tc.tile_pool(name="const", bufs=1))
            spool = ctx.enter_context(tc.tile_pool(name="stream", bufs=2))
            epool = ctx.enter_context(tc.tile_pool(name="e", bufs=2))
            wpool = ctx.enter_context(tc.tile_pool(name="w", bufs=2))
            tpool = ctx.enter_context(tc.tile_pool(name="t", bufs=2))
            rpool = ctx.enter_context(tc.tile_pool(name="r", bufs=2))
            vpool = ctx.enter_context(tc.tile_pool(name="v", bufs=3))
            opool = ctx.enter_context(tc.tile_pool(name="o", bufs=2))
            pspool = ctx.enter_context(
                tc.tile_pool(name="ps", bufs=2, space="PSUM")
            )
            vpspool = ctx.enter_context(
                tc.tile_pool(name="psv", bufs=2, space="PSUM")
            )

            # ---- resident inputs ----
            kw_sb = cpool.tile([128, C], F16)
            nc.sync.dma_start(kw_sb[:], kw_in[:])
            vw_sb = cpool.tile([128, C], F16)
            nc.sync.dma_start(vw_sb[:], vw_in[:])
            fw_sb = cpool.tile([128, C], F16)
            nc.sync.dma_start(fw_sb[:], fw_in[:])
            vbc_sb = cpool.tile([128, 1], F32)
            nc.sync.dma_start(vbc_sb[:], vbc_in[:])
            gammac_sb = cpool.tile([128, 1], F32)
            nc.sync.dma_start(gammac_sb[:], gammac_in[:])
            betac_sb = cpool.tile([128, 1], F32)
            nc.sync.dma_start(betac_sb[:], betac_in[:])
            fold128_sb = cpool.tile([128, C], F32)
            nc.sync.dma_start(fold128_sb[:], fold128_in[:])
            fold64_sb = cpool.tile([C, 128], F32)
            nc.sync.dma_start(fold64_sb[:], fold64_in[:])
            featT_sb = cpool.tile([128, HALF], F16)
            nc.sync.dma_start(featT_sb[:], featT_in[:])

            h_sb = cpool.tile([128, GROUPS, NG], F16)
            hpart = cpool.tile([128, GROUPS], F32)
            sqpart = cpool.tile([128, GROUPS], F32)
            junk = cpool.tile([128, NG], F16)

            # warm-up AllReduce: absorbs first-collective setup + skew
            warm_sb = cpool.tile([1, 1], F32)
            nc.gpsimd.memset(warm_sb[:], 0.0)
            nc.sync.dma_start(ccw_in[:], warm_sb[:])
            nc.gpsimd.collective_compute(
                "AllReduce",
                OP.add,
                replica_groups=[list(range(n_cores))],
                ins=[ccw_in[:]],
                outs=[ccw_out[:]],
            )

            def mm_pair(ps, w_sb_, fg_sb, jp):
                """two j-slots of a stacked projection into one psum tile"""
                for i in range(2):
                    j = jp * 2 + i
                    nc.tensor.matmul(
                        ps[0:C, i, :],
                        lhsT=w_sb_[0:C, :],
                        rhs=fg_sb[0:C, j, :],
                        tile_position=(0, 0),
                    )
                    nc.tensor.matmul(
                        ps[C:128, i, :],
                        lhsT=w_sb_[C:128, :],
                        rhs=fg_sb[C:128, j, :],
                        tile_position=(64, 64),
                    )

            # ---- main loop over groups ----
            for g in range(GROUPS):
                fg_sb = spool.tile([128, K, NG], F16, tag="fg")
                nc.sync.dma_start(fg_sb[:], fg_view[:, g, :, :])

                # e projection + exp, by pairs of j; sum-tree level 1
                # fires as soon as its quarter of e lands
                e_sb = epool.tile([128, K, NG], F16, tag="e")
                for jp in range(K // 2):
                    ps = pspool.tile([128, 2, NG], F32, tag="ps")
                    mm_pair(ps, kw_sb, fg_sb, jp)
                    nc.scalar.activation(
                        e_sb[:, jp * 2 : jp * 2 + 2, :],
                        ps[:],
                        AF.Exp,
                        scale=-0.125,
                    )
                # s = sum_K e: monolithic tree, collapsing in place in t1
                t1 = tpool.tile([128, K // 2, NG], F16, tag="t1")
                nc.vector.tensor_add(t1[:], e_sb[:, 0:8, :], e_sb[:, 8:16, :])
                nc.vector.tensor_add(
                    t1[:, 0:4, :], t1[:, 0:4, :], t1[:, 4:8, :]
                )
                nc.vector.tensor_add(
                    t1[:, 0:2, :], t1[:, 0:2, :], t1[:, 2:4, :]
                )
                nc.vector.tensor_add(
                    t1[:, 0:1, :], t1[:, 0:1, :], t1[:, 1:2, :]
                )
                # D = e - s
                w_sb = wpool.tile([128, K, NG], F16, tag="w")
                nc.vector.tensor_sub(
                    w_sb[:], e_sb[:], t1[:, 0:1, :].broadcast_to([128, K, NG])
                )

                # 1/s while v streams
                s32 = rpool.tile([128, NG], F32, tag="s32")
                nc.vector.tensor_copy(s32[:], t1[:, 0, :])
                r32 = rpool.tile([128, NG], F32, tag="r32")
                nc.vector.reciprocal_approx_fast(r32[:], s32[:])

                # v projection + W = (v + vb) * D (in place over D), then
                # the max-tree piece for each completed quarter
                for jp in range(K // 2):
                    ps = vpspool.tile([128, 2, NG], F32, tag="psv")
                    mm_pair(ps, vw_sb, fg_sb, jp)
                    wslice = w_sb[:, jp * 2 : jp * 2 + 2, :]
                    if W_MODES[jp] == "stt":
                        nc.vector.scalar_tensor_tensor(
                            wslice, ps[:], vbc_sb[:], wslice, OP.add, OP.mult
                        )
                    else:
                        v_sb = vpool.tile([128, 2, NG], F16, tag="v")
                        nc.scalar.activation(
                            v_sb[:], ps[:], AF.Identity, bias=vbc_sb[:]
                        )
                        eng = nc.gpsimd if W_MODES[jp] == "agps" else nc.vector
                        eng.tensor_mul(wslice, wslice, v_sb[:])
                # max_K W: monolithic tree, in place
                nc.vector.tensor_tensor(
                    w_sb[:, 0:8, :], w_sb[:, 0:8, :], w_sb[:, 8:16, :], OP.max
                )
                nc.vector.tensor_tensor(
                    w_sb[:, 0:4, :], w_sb[:, 0:4, :], w_sb[:, 4:8, :], OP.max
                )
                nc.vector.tensor_tensor(
                    w_sb[:, 0:2, :], w_sb[:, 0:2, :], w_sb[:, 2:4, :], OP.max
                )
                nc.vector.tensor_tensor(
                    w_sb[:, 0:1, :], w_sb[:, 0:1, :], w_sb[:, 1:2, :], OP.max
                )
                ctx_g = rpool.tile([128, NG], F16, tag="ctx")
                nc.vector.tensor_mul(ctx_g[:], w_sb[:, 0, :], r32[:])

                # phase 3 for this group: h = ctx@fw, BN partials
                hpt = vpspool.tile([128, 2, NG], F32, tag="psv")
                hps = hpt[:, 0, :]
                nc.tensor.matmul(
                    hps[0:C, :],
                    lhsT=fw_sb[0:C, :],
                    rhs=ctx_g[0:C, :],
                    tile_position=(0, 0),
                )
                nc.tensor.matmul(
                    hps[C:128, :],
                    lhsT=fw_sb[C:128, :],
                    rhs=ctx_g[C:128, :],
                    tile_position=(64, 64),
                )
                nc.scalar.activation(
                    h_sb[:, g, :],
                    hps[:],
                    AF.Copy,
                    accum_out=hpart[:, g : g + 1],
                )
                nc.scalar.activation(
                    junk[:],
                    hps[:],
                    AF.Square,
                    accum_out=sqpart[:, g : g + 1],
                )

            # ---- BN stats: allreduce sum/sumsq, fold A/B halves ----
            stat_sb = cpool.tile([128, 2], F32)
            nc.vector.tensor_reduce(
                stat_sb[:, 0:1], hpart[:], axis=AX.X, op=OP.add
            )
            nc.vector.tensor_reduce(
                stat_sb[:, 1:2], sqpart[:], axis=AX.X, op=OP.add
            )
            # collective payload [stat | stat partition-swapped]: after the
            # AllReduce one DVE add folds the A/B halves (partition p and
            # p+64 hold the same channel), replacing PE fold matmuls
            nc.sync.dma_start(cc_in[:, 0:2], stat_sb[:])
            nc.sync.dma_start(cc_in[0:C, 2:4], stat_sb[C:128, :])
            nc.sync.dma_start(cc_in[C:128, 2:4], stat_sb[0:C, :])
            nc.gpsimd.collective_compute(
                "AllReduce",
                OP.add,
                replica_groups=[list(range(n_cores))],
                ins=[cc_in[:]],
                outs=[cc_out[:]],
            )
            stat2_sb = cpool.tile([128, 4], F32)
            nc.sync.dma_start(stat2_sb[:], cc_out[:])
            stf_sb = cpool.tile([128, 2], F32)
            nc.vector.tensor_add(
                stf_sb[:], stat2_sb[:, 0:2], stat2_sb[:, 2:4]
            )

            # mean = s/M ; var = sq/M - mean^2 ; scale = gamma/sqrt(var+eps)
            # bias = beta - mean*scale
            mean_sb = cpool.tile([128, 1], F32)
            nc.scalar.mul(mean_sb[:], stf_sb[:, 0:1], 1.0 / M_TOT)
            var_sb = cpool.tile([128, 1], F32)
            nc.scalar.mul(var_sb[:], stf_sb[:, 1:2], 1.0 / M_TOT)
            mu2_sb = cpool.tile([128, 1], F32)
            nc.scalar.activation(mu2_sb[:], mean_sb[:], AF.Square)
            nc.vector.tensor_sub(var_sb[:], var_sb[:], mu2_sb[:])
            eps_sb = cpool.tile([128, 1], F32)
            nc.gpsimd.memset(eps_sb[:], EPS)
            sq_sb = cpool.tile([128, 1], F32)
            nc.scalar.activation(sq_sb[:], var_sb[:], AF.Sqrt, bias=eps_sb[:])
            rstd_sb = cpool.tile([128, 1], F32)
            nc.vector.reciprocal(rstd_sb[:], sq_sb[:])
            scale_sb = cpool.tile([128, 1], F32)
            nc.vector.tensor_mul(scale_sb[:], rstd_sb[:], gammac_sb[:])
            bias_sb = cpool.tile([128, 1], F32)
            nc.vector.tensor_mul(bias_sb[:], mean_sb[:], scale_sb[:])
            nc.vector.tensor_sub(bias_sb[:], betac_sb[:], bias_sb[:])

            # ---- epilogue: bn-affine (Act), lrelu (DVE), residual (gps) ----
            OCH = 1024
            for ch in range(HALF // OCH):
                hv = h_sb[:].rearrange("p a b -> p (a b)")[
                    :, ch * OCH : (ch + 1) * OCH
                ]
                bn_sb = opool.tile([128, OCH], F16, tag="bn")
                nc.scalar.activation(
                    bn_sb[:], hv, AF.Identity, bias=bias_sb[:], scale=scale_sb[:]
                )
                lr_sb = opool.tile([128, OCH], F16, tag="lr")
                nc.vector.scalar_tensor_tensor(
                    lr_sb[:], bn_sb[:], ALPHA, bn_sb[:], OP.mult, OP.max
                )
                out_sb = opool.tile([128, OCH], F16, tag="out")
                nc.vector.tensor_add(
                    out_sb[:], lr_sb[:], featT_sb[:, ch * OCH : (ch + 1) * OCH]
                )
                nc.sync.dma_start(
                    out_dram[:, ch * OCH : (ch + 1) * OCH], out_sb[:]
                )

    nc.compile()
    return nc


_PROG = None


def _get_program():
    global _PROG
    if _PROG is None:
        _PROG = _build_program()
    return _PROG


def _prep_core_inputs(f16b, idx, consts, core):
    b, h = core // 2, core % 2
    f16 = f16b[b]                                        # [N, C] fp16
    il = np.asarray(idx[b, h * NODES : (h + 1) * NODES])  # [NODES, K] int
    # fg[p, g, j, n]: p<64 -> channel p of half-A col (g*512+n, slot j)
    fg = np.empty((128, GROUPS, K, NG), np.float16)
    for half in range(2):
        ih = il[half * HALF : (half + 1) * HALF]          # [HALF, K]
        g_ = f16[ih]                                      # [HALF, K, C]
        g_ = g_.reshape(GROUPS, NG, K, C).transpose(3, 0, 2, 1)
        fg[half * C : (half + 1) * C] = g_
    featT = np.empty((128, HALF), np.float16)
    base = h * NODES
    featT[0:C] = f16[base : base + HALF].T
    featT[C:128] = f16[base + HALF : base + NODES].T
    return {
        "fg": fg.reshape(128, GROUPS * K * NG),
        "featT": featT,
        **consts,
    }


def _make_in_maps(features, idx, kw, vw, vb, fw, gamma, beta):
    f16b = np.asarray(features, np.float32).astype(np.float16)
    stack2 = lambda x: np.tile(
        np.asarray(x, np.float32).reshape(C), 2
    ).reshape(128, 1)
    fold128 = np.zeros((128, C), np.float32)
    fold128[np.arange(128), np.arange(128) % C] = 1.0
    fold64 = np.ascontiguousarray(fold128.T)
    consts = {
        "kw": np.tile(np.asarray(kw, np.float16), (2, 1)),
        "vw": np.tile(np.asarray(vw, np.float16), (2, 1)),
        "fw": np.tile(np.asarray(fw, np.float16), (2, 1)),
        "vbc": stack2(vb),
        "gammac": stack2(gamma),
        "betac": stack2(beta),
        "fold128": fold128,
        "fold64": fold64,
    }
    idx = np.asarray(idx)
    return [
        _prep_core_inputs(f16b, idx, consts, c) for c in range(N_CORES)
    ]


def kernel(features, pos, qw, qb, kw, kb, vw, vb, fw, fb, gamma, beta, idx):
    del pos, qw, qb, kb, fb  # drop out of the math exactly (see docstring)
    nc = _get_program()
    in_maps = _make_in_maps(features, idx, kw, vw, vb, fw, gamma, beta)
    res = run_bass_kernel_spmd(nc, in_maps, list(range(N_CORES)))

    out = np.empty((B, N, C), np.float32)
    for c in range(N_CORES):
        b, h = c // 2, c % 2
        o = np.asarray(res.results[c]["out"], np.float32)  # [128, HALF]
        base = h * NODES
        out[b, base : base + HALF] = o[0:C].T
        out[b, base + HALF : base + NODES] = o[C:128].T
    return out
